# revision 9
# baseline (speedup 1.0000x reference)
"""LSTM cell (batch 8192, input 512, hidden 512) on 8 Trainium2 NeuronCores.

Data-parallel over the batch dim: each core handles 1024 rows. Weights are
replicated. The host pre-transposes both matmul operands so the contraction
dim (fan_in = 1024) lands on SBUF partitions:

  gate.T[n, b] = sum_k W.T[k, n] * combined.T[k, b]     (matmul: lhsT.T @ rhs)

so the kernel computes everything in [hidden, batch] layout; gate biases
become per-partition vectors (free on the ACT activation op), and the host
transposes the outputs back after the gather.

Matmul operands are cast to bf16 on the host (f32 matmul on PE is 4x slower
per the cost model); accumulation is f32 in PSUM; the elementwise tail runs
f32 with c_prev/c_next/h_next stored bf16 (host upcasts after the gather).

Schedule notes (from perfetto iterations):
- The PE is the bottleneck: 256 matmuls x 216ns = 55.3us at full clock plus
  ~6us of fixed preamble + DMA-launch latency and ~6us of tail+teardown.
- The Tensor engine ramps 0.65 -> 1.2 -> 2.4 GHz over several us of
  continuous execution; dummy matmuls on a memset scratch tile burn the
  unavoidable DMA-launch wait finishing most of the ramp off the critical
  path (worth ~1.5-2us).
- Each engine DGE ring drains in-order at only ~110-150 GB/s, so h=0's
  input set (3 MB consumed in ~13us) is split between the sync and scalar
  rings in need-time order. The gpsimd ring carries ONLY outputs: mixing
  inputs onto it queues h=0 weight strips behind output transfers and cost
  a 5us PE stall in one iteration.
- The first matmul's two deps (w k=0 strip, xh k=0 b2=0 half) each lead a
  different ring; k=0 runs b2=0's four gates before b2=1's to match the
  half-tile arrival order.
"""

import numpy as np

import concourse.bacc as bacc
import concourse.bass as bass
import concourse.mybir as mybir
from concourse import tile
from concourse.bass_utils import run_bass_kernel_spmd

N_CORES = 8
BATCH = 8192
B = BATCH // N_CORES  # 1024 batch rows per core
K = 1024              # fan_in = input_dim + hidden_dim
H = 512               # hidden dim
NG = 4                # gates: i, f, c, o
KT = K // 128         # 8 contraction tiles
HT = H // 128         # 4 hidden chunks per gate
BT = B // 512         # 2 batch halves (PSUM free-dim limit is 512 f32)

MM_DT = mybir.dt.bfloat16
F32 = mybir.dt.float32
BF16 = mybir.dt.bfloat16

_SIG = mybir.ActivationFunctionType.Sigmoid
_TANH = mybir.ActivationFunctionType.Tanh
# gate order within the concatenated weight: i, f, c, o
_GATE_FN = [_SIG, _SIG, _TANH, _SIG]


def _build():
    nc = bacc.Bacc(
        "TRN2",
        target_bir_lowering=False,
        debug=False,
        num_devices=N_CORES,
    )

    xhT = nc.dram_tensor("xhT", [K, B], MM_DT, kind="ExternalInput")
    # wTh column order is h-major: [h, g, p] -> col h*512 + g*128 + p, so the
    # h=0 slice of every k-tile is one contiguous 512-col strip.
    wTh = nc.dram_tensor("wTh", [K, NG * H], MM_DT, kind="ExternalInput")
    bias2d = nc.dram_tensor("bias2d", [128, NG * HT], F32, kind="ExternalInput")
    c_prevT = nc.dram_tensor("c_prevT", [H, B], BF16, kind="ExternalInput")
    h_nextT = nc.dram_tensor("h_nextT", [H, B], BF16, kind="ExternalOutput")
    c_nextT = nc.dram_tensor("c_nextT", [H, B], BF16, kind="ExternalOutput")

    with tile.TileContext(nc) as tc:
        with (
            tc.tile_pool(name="wts", bufs=1) as wpool,
            tc.tile_pool(name="acts", bufs=1) as apool,
            tc.tile_pool(name="cprev", bufs=1) as cpool,
            tc.tile_pool(name="gates", bufs=3) as gpool,
            tc.tile_pool(name="ew", bufs=3) as epool,
            tc.tile_pool(name="psum", bufs=1, space="PSUM") as pspool,
        ):
            xh_tiles = [None] * KT      # k -> [128, 1024] (k=0: two halves)
            xh0_half = [None, None]     # b2 -> [128, 512]
            cp_tiles = [None] * HT
            wt_tiles = [[None] * HT for _ in range(KT)]  # [k][h] -> [128, 512]

            def _load_xh(k, eng):
                xt = apool.tile([128, B], MM_DT, tag=f"xh{k}", name=f"xh{k}")
                eng.dma_start(xt[:], xhT[k * 128:(k + 1) * 128, :])
                xh_tiles[k] = xt

            def _load_w(k, h, eng):
                wt = wpool.tile([128, NG * 128], MM_DT, tag=f"w{k}_{h}",
                                name=f"w{k}_{h}")
                eng.dma_start(
                    wt[:], wTh[k * 128:(k + 1) * 128, h * 512:(h + 1) * 512]
                )
                wt_tiles[k][h] = wt

            def _load_cp(h, eng):
                ct = cpool.tile([128, B], BF16, tag=f"cp{h}", name=f"cp{h}")
                eng.dma_start(ct[:], c_prevT[h * 128:(h + 1) * 128, :])
                cp_tiles[h] = ct

            # --- PE pstate warmup scratch (memset on gpsimd: it starts
            # earliest and the 0.1us memset barely delays its ring) ---------
            warm_src = apool.tile([128, 384], MM_DT, tag="warm_src", name="warm_src")
            nc.gpsimd.memset(warm_src[:], 0.5)

            # --- input DMA schedule: sync + scalar rings, need-time order --
            # (gpsimd stays output-only; bias is tiny and ahead of outputs)
            _load_w(0, 0, nc.scalar)
            for b2 in range(BT):
                xt = apool.tile([128, 512], MM_DT, tag=f"xh0_{b2}",
                                name=f"xh0_{b2}")
                nc.sync.dma_start(xt[:], xhT[0:128, b2 * 512:(b2 + 1) * 512])
                xh0_half[b2] = xt
            bias_t = wpool.tile([128, NG * HT], F32, tag="bias", name="bias_t")
            nc.gpsimd.dma_start(bias_t[:], bias2d[:])
            _load_xh(1, nc.scalar)
            _load_w(1, 0, nc.sync)
            _load_w(2, 0, nc.scalar)
            _load_xh(2, nc.sync)
            _load_xh(3, nc.scalar)
            _load_w(3, 0, nc.sync)
            _load_w(4, 0, nc.scalar)
            _load_xh(4, nc.sync)
            _load_xh(5, nc.scalar)
            _load_w(5, 0, nc.sync)
            _load_w(6, 0, nc.scalar)
            _load_xh(6, nc.sync)
            _load_xh(7, nc.scalar)
            _load_w(7, 0, nc.sync)
            # h>=1 strips + c_prev: alternate the two input rings
            for h in range(1, HT):
                for k in range(KT):
                    _load_w(k, h, nc.scalar if k % 2 == 0 else nc.sync)
                _load_cp(h - 1, nc.sync if h % 2 else nc.scalar)
            _load_cp(HT - 1, nc.scalar)

            def _rhs(k, b2):
                if k == 0:
                    return xh0_half[b2][:]
                return xh_tiles[k][:, b2 * 512:(b2 + 1) * 512]

            def _lhsT(k, h, g):
                return wt_tiles[k][h][:, g * 128:(g + 1) * 128]

            def _mk_psum(g, h, b2):
                return pspool.tile(
                    [128, 512], F32,
                    tag=f"ps{g}_{b2 % 2}", name=f"ps{g}_{h}_{b2}",
                )

            # --- PE pstate warmup matmuls ---------------------------------
            # Sized to end right as the first real matmul's DMAs land
            # (~9.5-10us); the free-64 tail keeps the handoff granular.
            # Results land in the ps3_1 bank and are overwritten by its
            # start=True matmul; nothing reads them.
            warm_ps = pspool.tile([128, 512], F32, tag="ps3_1", name="warm_ps")
            for i in range(13):
                nc.tensor.matmul(
                    warm_ps[:, :256], warm_src[:, :128], warm_src[:, 128:],
                    start=True, stop=True,
                )
            for i in range(4):
                nc.tensor.matmul(
                    warm_ps[:, :64], warm_src[:, :128], warm_src[:, 128:192],
                    start=True, stop=True,
                )

            def _elementwise(h, b2, psum, last=False):
                """Activations + LSTM cell tail for one (h, b2) group."""
                hs = slice(h * 128, (h + 1) * 128)
                cs = slice(b2 * 512, (b2 + 1) * 512)

                def _act_gate(g):
                    t = gpool.tile(
                        [128, 512], F32, tag=f"g{g}", name=f"g{g}_{h}_{b2}",
                    )
                    nc.scalar.activation(
                        t[:], psum[g][:], _GATE_FN[g],
                        bias=bias_t[:, g * HT + h:g * HT + h + 1],
                    )
                    return t

                # i, f, c~ first; the whole c_next/tanh chain runs while the
                # output gate's matmuls are still on the PE (gate-major issue
                # order puts o last).
                gi = _act_gate(0)
                gf = _act_gate(1)
                gc = _act_gate(2)

                t1 = epool.tile([128, 512], F32, tag="t1", name=f"t1_{h}_{b2}")
                nc.vector.tensor_mul(t1[:], gi[:], gc[:])       # i * c~
                t2 = epool.tile([128, 512], F32, tag="t2", name=f"t2_{h}_{b2}")
                nc.vector.tensor_mul(t2[:], gf[:], cp_tiles[h][:, cs])
                cn = epool.tile([128, 512], BF16, tag="cn", name=f"cn_{h}_{b2}")
                nc.vector.tensor_add(cn[:], t1[:], t2[:])
                # last group: cn on sync, hn on scalar -- two rings generate
                # descriptors in parallel so the final transfer isn't queued
                # behind the previous one's 0.7us DIRECT2D.
                (nc.sync if last else nc.gpsimd).dma_start(c_nextT[hs, cs], cn[:])

                th = epool.tile([128, 512], F32, tag="th", name=f"th_{h}_{b2}")
                nc.scalar.activation(th[:], cn[:], _TANH)

                go = _act_gate(3)
                hn = epool.tile([128, 512], BF16, tag="hn", name=f"hn_{h}_{b2}")
                nc.vector.tensor_mul(hn[:], go[:], th[:])
                (nc.scalar if last else nc.gpsimd).dma_start(h_nextT[hs, cs], hn[:])

            # h=0 rides the input-DMA ramp: all 8 PSUM banks (4 gates x 2
            # batch halves), k-major so the PE consumes each k-tile 8
            # matmuls at a time, giving the two input rings slack to stay
            # ahead. k=0 runs b2=0's gates before b2=1's to match the
            # half-tile arrival order.
            psum0 = {b2: [_mk_psum(g, 0, b2) for g in range(NG)] for b2 in range(BT)}
            for k in range(KT):
                if k == 0:
                    for b2 in range(BT):
                        for g in range(NG):
                            nc.tensor.matmul(
                                psum0[b2][g][:], _lhsT(0, 0, g), _rhs(0, b2),
                                start=True, stop=False,
                            )
                else:
                    for g in range(NG):
                        for b2 in range(BT):
                            nc.tensor.matmul(
                                psum0[b2][g][:], _lhsT(k, 0, g), _rhs(k, b2),
                                start=False, stop=(k == KT - 1),
                            )
            for b2 in range(BT):
                _elementwise(0, b2, psum0[b2])

            # h>=1: inputs are resident; per-(h,b2) 4-bank groups with b2
            # parity alternating between the two bank sets, so each set's
            # ACT drain overlaps the other's matmuls.
            for h in range(1, HT):
                for b2 in range(BT):
                    psum = [_mk_psum(g, h, b2) for g in range(NG)]
                    # gate-major, output gate (g=3) last: everything except
                    # ACT(o) and h=o*tanh(c) drains while o's matmuls run.
                    for g in range(NG):
                        for k in range(KT):
                            nc.tensor.matmul(
                                psum[g][:],
                                _lhsT(k, h, g),
                                _rhs(k, b2),
                                start=(k == 0),
                                stop=(k == KT - 1),
                            )
                    last = (h == HT - 1 and b2 == BT - 1)
                    _elementwise(h, b2, psum, last=last)

    nc.compile()
    return nc


_NC_CACHE = None
_LAST_IN_MAPS = None


def kernel(x, h_prev, c_prev, W_i, b_i, W_f, b_f, W_c, b_c, W_o, b_o):
    global _NC_CACHE, _LAST_IN_MAPS
    if _NC_CACHE is None:
        _NC_CACHE = _build()
    nc = _NC_CACHE

    np_bf16 = mybir.dt.np(MM_DT)

    combT = np.concatenate([x, h_prev], axis=1).T          # (K, BATCH) f32
    combT = combT.astype(np_bf16)
    wT = np.concatenate([W_i, W_f, W_c, W_o], axis=0).T    # (K, 4H): col g*H+h*128+p
    # h-major column order: col h*512 + g*128 + p  (see _build)
    wTh = np.ascontiguousarray(
        wT.reshape(K, NG, HT, 128).transpose(0, 2, 1, 3).reshape(K, NG * H)
    ).astype(np_bf16)
    bias2d = np.ascontiguousarray(
        np.concatenate([b_i, b_f, b_c, b_o]).reshape(NG * HT, 128).T
    ).astype(np.float32)                                   # (128, 16)
    c_prevT = c_prev.T.astype(np_bf16)                     # (H, BATCH)

    in_maps = []
    for j in range(N_CORES):
        cols = slice(j * B, (j + 1) * B)
        in_maps.append({
            "xhT": np.ascontiguousarray(combT[:, cols]),
            "wTh": wTh,
            "bias2d": bias2d,
            "c_prevT": np.ascontiguousarray(c_prevT[:, cols]),
        })

    _LAST_IN_MAPS = in_maps
    try:
        res = run_bass_kernel_spmd(nc, in_maps, core_ids=list(range(N_CORES)))
    except Exception:
        # transient NRT_EXEC_UNIT_UNRECOVERABLE has been observed once on an
        # otherwise-correct NEFF; one retry is cheap insurance.
        res = run_bass_kernel_spmd(nc, in_maps, core_ids=list(range(N_CORES)))

    h_next = np.concatenate([r["h_nextT"].T for r in res.results], axis=0)
    c_next = np.concatenate([r["c_nextT"].T for r in res.results], axis=0)
    return (h_next.astype(np.float32), c_next.astype(np.float32))


# revision 10
# speedup vs baseline: 1.1596x; 1.1596x over previous
"""LSTM cell (batch 8192, input 512, hidden 512) on 8 Trainium2 NeuronCores.

Data-parallel over the batch dim: each core handles 1024 rows. Weights are
replicated. The host pre-transposes both matmul operands so the contraction
dim (fan_in = 1024) lands on SBUF partitions:

  gate.T[n, b] = sum_k W.T[k, n] * combined.T[k, b]     (matmul: lhsT.T @ rhs)

so the kernel computes everything in [hidden, batch] layout; gate biases
become per-partition vectors (free on the ACT activation op), and the host
transposes the outputs back after the gather.

Matmul operands are cast to bf16 on the host (f32 matmul on PE is 4x slower
per the cost model); accumulation is f32 in PSUM; the elementwise tail runs
f32 with c_prev/c_next/h_next stored bf16 (host upcasts after the gather).

Schedule notes (from perfetto iterations):
- The PE is the bottleneck: 256 matmuls x 216ns = 55.3us at full clock plus
  ~6us of fixed preamble + DMA-launch latency and ~6us of tail+teardown.
- The Tensor engine ramps 0.65 -> 1.2 -> 2.4 GHz over several us of
  continuous execution; dummy matmuls on a memset scratch tile burn the
  unavoidable DMA-launch wait finishing most of the ramp off the critical
  path (worth ~1.5-2us).
- Each engine DGE ring drains in-order at only ~110-150 GB/s, so h=0's
  input set (3 MB consumed in ~13us) is split between the sync and scalar
  rings in need-time order. The gpsimd ring carries ONLY outputs: mixing
  inputs onto it queues h=0 weight strips behind output transfers and cost
  a 5us PE stall in one iteration.
- The first matmul's two deps (w k=0 strip, xh k=0 b2=0 half) each lead a
  different ring; k=0 runs b2=0's four gates before b2=1's to match the
  half-tile arrival order.
"""

import numpy as np

import concourse.bacc as bacc
import concourse.bass as bass
import concourse.mybir as mybir
from concourse import tile
from concourse.bass_utils import run_bass_kernel_spmd

N_CORES = 8
BATCH = 8192
B = BATCH // N_CORES  # 1024 batch rows per core
K = 1024              # fan_in = input_dim + hidden_dim
H = 512               # hidden dim
NG = 4                # gates: i, f, c, o
KT = K // 128         # 8 contraction tiles
HT = H // 128         # 4 hidden chunks per gate
BT = B // 512         # 2 batch halves (PSUM free-dim limit is 512 f32)

MM_DT = mybir.dt.bfloat16
F32 = mybir.dt.float32
BF16 = mybir.dt.bfloat16

_SIG = mybir.ActivationFunctionType.Sigmoid
_TANH = mybir.ActivationFunctionType.Tanh
# gate order within the concatenated weight: i, f, c, o
_GATE_FN = [_SIG, _SIG, _TANH, _SIG]


def _build():
    nc = bacc.Bacc(
        "TRN2",
        target_bir_lowering=False,
        debug=False,
        num_devices=N_CORES,
    )

    xhT = nc.dram_tensor("xhT", [K, B], MM_DT, kind="ExternalInput")
    # wTh column order is h-major: [h, g, p] -> col h*512 + g*128 + p, so the
    # h=0 slice of every k-tile is one contiguous 512-col strip.
    wTh = nc.dram_tensor("wTh", [K, NG * H], MM_DT, kind="ExternalInput")
    bias2d = nc.dram_tensor("bias2d", [128, NG * HT], F32, kind="ExternalInput")
    c_prevT = nc.dram_tensor("c_prevT", [H, B], BF16, kind="ExternalInput")
    h_nextT = nc.dram_tensor("h_nextT", [H, B], BF16, kind="ExternalOutput")
    c_nextT = nc.dram_tensor("c_nextT", [H, B], BF16, kind="ExternalOutput")

    with tile.TileContext(nc) as tc:
        with (
            tc.tile_pool(name="wts", bufs=1) as wpool,
            tc.tile_pool(name="acts", bufs=1) as apool,
            tc.tile_pool(name="cprev", bufs=1) as cpool,
            tc.tile_pool(name="gates", bufs=3) as gpool,
            tc.tile_pool(name="ew", bufs=3) as epool,
            tc.tile_pool(name="psum", bufs=1, space="PSUM") as pspool,
        ):
            xh_tiles = [None] * KT      # k -> [128, 1024] (k=0: two halves)
            xh0_half = [None, None]     # b2 -> [128, 512]
            cp_tiles = [None] * HT
            wt_tiles = [[None] * HT for _ in range(KT)]  # [k][h] -> [128, 512]

            def _load_xh(k, eng):
                xt = apool.tile([128, B], MM_DT, tag=f"xh{k}", name=f"xh{k}")
                eng.dma_start(xt[:], xhT[k * 128:(k + 1) * 128, :])
                xh_tiles[k] = xt

            def _load_w(k, h, eng):
                wt = wpool.tile([128, NG * 128], MM_DT, tag=f"w{k}_{h}",
                                name=f"w{k}_{h}")
                eng.dma_start(
                    wt[:], wTh[k * 128:(k + 1) * 128, h * 512:(h + 1) * 512]
                )
                wt_tiles[k][h] = wt

            def _load_cp(h, eng):
                ct = cpool.tile([128, B], BF16, tag=f"cp{h}", name=f"cp{h}")
                eng.dma_start(ct[:], c_prevT[h * 128:(h + 1) * 128, :])
                cp_tiles[h] = ct

            # --- PE pstate warmup scratch (memset on gpsimd: it starts
            # earliest and the 0.1us memset barely delays its ring) ---------
            warm_src = apool.tile([128, 384], MM_DT, tag="warm_src", name="warm_src")
            nc.gpsimd.memset(warm_src[:], 0.5)

            # --- input DMA schedule ---------------------------------------
            # ALL bulk input goes on the sync ring in exact consumption
            # order: its sequencer has no other duties, and one ring already
            # sustains ~280 GB/s -- the shared 16-engine backend is the real
            # limit, so splitting rings adds no bandwidth. The scalar ring
            # carries ONLY the two k<2 weight strips issued before any ACT
            # exists: bulk DMA there blocks the scalar sequencer in
            # DIRECT2D backpressure and starves ACT issue (8us PE stall).
            _load_w(0, 0, nc.scalar)
            for b2 in range(BT):
                xt = apool.tile([128, 512], MM_DT, tag=f"xh0_{b2}",
                                name=f"xh0_{b2}")
                nc.sync.dma_start(xt[:], xhT[0:128, b2 * 512:(b2 + 1) * 512])
                xh0_half[b2] = xt
            bias_t = wpool.tile([128, NG * HT], F32, tag="bias", name="bias_t")
            nc.gpsimd.dma_start(bias_t[:], bias2d[:])
            _load_w(1, 0, nc.scalar)
            _load_xh(1, nc.sync)
            for k in range(2, KT):
                _load_w(k, 0, nc.sync)
                _load_xh(k, nc.sync)
            # h>=1 strips; c_prev rides after each h's strips (cp{h} isn't
            # needed until h's ACT drain, well after h's matmuls)
            for h in range(1, HT):
                for k in range(KT):
                    _load_w(k, h, nc.sync)
                _load_cp(h - 1, nc.sync)
            _load_cp(HT - 1, nc.sync)

            def _rhs(k, b2):
                if k == 0:
                    return xh0_half[b2][:]
                return xh_tiles[k][:, b2 * 512:(b2 + 1) * 512]

            def _lhsT(k, h, g):
                return wt_tiles[k][h][:, g * 128:(g + 1) * 128]

            def _mk_psum(g, h, b2):
                return pspool.tile(
                    [128, 512], F32,
                    tag=f"ps{g}_{b2 % 2}", name=f"ps{g}_{h}_{b2}",
                )

            # --- PE pstate warmup matmuls ---------------------------------
            # Sized to end right as the first real matmul's DMAs land
            # (~9.5-10us); the free-64 tail keeps the handoff granular.
            # Results land in the ps3_1 bank and are overwritten by its
            # start=True matmul; nothing reads them.
            warm_ps = pspool.tile([128, 512], F32, tag="ps3_1", name="warm_ps")
            for i in range(13):
                nc.tensor.matmul(
                    warm_ps[:, :256], warm_src[:, :128], warm_src[:, 128:],
                    start=True, stop=True,
                )
            for i in range(4):
                nc.tensor.matmul(
                    warm_ps[:, :64], warm_src[:, :128], warm_src[:, 128:192],
                    start=True, stop=True,
                )

            def _elementwise(h, b2, psum, last=False):
                """Activations + LSTM cell tail for one (h, b2) group."""
                hs = slice(h * 128, (h + 1) * 128)
                cs = slice(b2 * 512, (b2 + 1) * 512)

                def _act_gate(g):
                    t = gpool.tile(
                        [128, 512], F32, tag=f"g{g}", name=f"g{g}_{h}_{b2}",
                    )
                    nc.scalar.activation(
                        t[:], psum[g][:], _GATE_FN[g],
                        bias=bias_t[:, g * HT + h:g * HT + h + 1],
                    )
                    return t

                # i, f, c~ first; the whole c_next/tanh chain runs while the
                # output gate's matmuls are still on the PE (gate-major issue
                # order puts o last).
                gi = _act_gate(0)
                gf = _act_gate(1)
                gc = _act_gate(2)

                t1 = epool.tile([128, 512], F32, tag="t1", name=f"t1_{h}_{b2}")
                nc.vector.tensor_mul(t1[:], gi[:], gc[:])       # i * c~
                t2 = epool.tile([128, 512], F32, tag="t2", name=f"t2_{h}_{b2}")
                nc.vector.tensor_mul(t2[:], gf[:], cp_tiles[h][:, cs])
                cn = epool.tile([128, 512], BF16, tag="cn", name=f"cn_{h}_{b2}")
                nc.vector.tensor_add(cn[:], t1[:], t2[:])
                # last group: cn on sync, hn on scalar -- two rings generate
                # descriptors in parallel so the final transfer isn't queued
                # behind the previous one's 0.7us DIRECT2D.
                (nc.sync if last else nc.gpsimd).dma_start(c_nextT[hs, cs], cn[:])

                th = epool.tile([128, 512], F32, tag="th", name=f"th_{h}_{b2}")
                nc.scalar.activation(th[:], cn[:], _TANH)

                go = _act_gate(3)
                hn = epool.tile([128, 512], BF16, tag="hn", name=f"hn_{h}_{b2}")
                nc.vector.tensor_mul(hn[:], go[:], th[:])
                (nc.scalar if last else nc.gpsimd).dma_start(h_nextT[hs, cs], hn[:])

            # h=0 rides the input-DMA ramp: all 8 PSUM banks (4 gates x 2
            # batch halves), k-major so the PE consumes each k-tile 8
            # matmuls at a time, giving the two input rings slack to stay
            # ahead. k=0 runs b2=0's gates before b2=1's to match the
            # half-tile arrival order.
            psum0 = {b2: [_mk_psum(g, 0, b2) for g in range(NG)] for b2 in range(BT)}
            for k in range(KT):
                if k == 0:
                    for b2 in range(BT):
                        for g in range(NG):
                            nc.tensor.matmul(
                                psum0[b2][g][:], _lhsT(0, 0, g), _rhs(0, b2),
                                start=True, stop=False,
                            )
                else:
                    for g in range(NG):
                        for b2 in range(BT):
                            nc.tensor.matmul(
                                psum0[b2][g][:], _lhsT(k, 0, g), _rhs(k, b2),
                                start=False, stop=(k == KT - 1),
                            )
            for b2 in range(BT):
                _elementwise(0, b2, psum0[b2])

            # h>=1: inputs are resident; per-(h,b2) 4-bank groups with b2
            # parity alternating between the two bank sets, so each set's
            # ACT drain overlaps the other's matmuls.
            for h in range(1, HT):
                for b2 in range(BT):
                    psum = [_mk_psum(g, h, b2) for g in range(NG)]
                    # gate-major, output gate (g=3) last: everything except
                    # ACT(o) and h=o*tanh(c) drains while o's matmuls run.
                    for g in range(NG):
                        for k in range(KT):
                            nc.tensor.matmul(
                                psum[g][:],
                                _lhsT(k, h, g),
                                _rhs(k, b2),
                                start=(k == 0),
                                stop=(k == KT - 1),
                            )
                    last = (h == HT - 1 and b2 == BT - 1)
                    _elementwise(h, b2, psum, last=last)

    nc.compile()
    return nc


_NC_CACHE = None
_LAST_IN_MAPS = None


def kernel(x, h_prev, c_prev, W_i, b_i, W_f, b_f, W_c, b_c, W_o, b_o):
    global _NC_CACHE, _LAST_IN_MAPS
    if _NC_CACHE is None:
        _NC_CACHE = _build()
    nc = _NC_CACHE

    np_bf16 = mybir.dt.np(MM_DT)

    combT = np.concatenate([x, h_prev], axis=1).T          # (K, BATCH) f32
    combT = combT.astype(np_bf16)
    wT = np.concatenate([W_i, W_f, W_c, W_o], axis=0).T    # (K, 4H): col g*H+h*128+p
    # h-major column order: col h*512 + g*128 + p  (see _build)
    wTh = np.ascontiguousarray(
        wT.reshape(K, NG, HT, 128).transpose(0, 2, 1, 3).reshape(K, NG * H)
    ).astype(np_bf16)
    bias2d = np.ascontiguousarray(
        np.concatenate([b_i, b_f, b_c, b_o]).reshape(NG * HT, 128).T
    ).astype(np.float32)                                   # (128, 16)
    c_prevT = c_prev.T.astype(np_bf16)                     # (H, BATCH)

    in_maps = []
    for j in range(N_CORES):
        cols = slice(j * B, (j + 1) * B)
        in_maps.append({
            "xhT": np.ascontiguousarray(combT[:, cols]),
            "wTh": wTh,
            "bias2d": bias2d,
            "c_prevT": np.ascontiguousarray(c_prevT[:, cols]),
        })

    _LAST_IN_MAPS = in_maps
    try:
        res = run_bass_kernel_spmd(nc, in_maps, core_ids=list(range(N_CORES)))
    except Exception:
        # transient NRT_EXEC_UNIT_UNRECOVERABLE has been observed once on an
        # otherwise-correct NEFF; one retry is cheap insurance.
        res = run_bass_kernel_spmd(nc, in_maps, core_ids=list(range(N_CORES)))

    h_next = np.concatenate([r["h_nextT"].T for r in res.results], axis=0)
    c_next = np.concatenate([r["c_nextT"].T for r in res.results], axis=0)
    return (h_next.astype(np.float32), c_next.astype(np.float32))
